# revision 29
# baseline (speedup 1.0000x reference)
"""Trainium2 Bass kernel for nn_PositionalEmbedding (embedding-lookup form).

Math: out[b, 2j]   = mean_k sin(params[k] * dc[b,k] * inv_freq[j])
      out[b, 2j+1] = mean_k cos(params[k] * dc[b,k] * inv_freq[j])

dc[b,k] are integers in [0, 60), so the whole map is out = OH @ T where
OH[b, (k,v)] is a 360-col one-hot (6 ones per row) and T[(k,v), f] the
sin/cos dictionary.  T is numerically low rank (the small inv_freq columns
are near-polynomial in v), so the host factors T = A @ Bm (SVD, r=128,
tail ~1e-12) and the device computes

    G^T[r, b]  = sum_c A_c^T @ OH_c          (3 matmuls per 512-col group)
    out[b, f]  = (G^T slice)^T @ Bm          (4 matmuls, K=r, per group)

which runs the 360-deep contraction once per 512 batch columns instead of
once per 128-row tile (the baseline), cutting PE streaming ~40% and making
the kernel output-DMA bound (34 MB fp32 @ ~358 GB/s/core ≈ 94 us).

One-hot build per group: 3 replication matmuls (K=6) into 3 adjacent PSUM
banks + two fused DVE is_equal ops (chunks 0-1, then 2; the per-partition
scalar v = p%60 is the same for all chunks).  A ~5 us dummy-matmul warm-up
plus filler matmuls keep the PE HAM clock gate at K=8/8 throughout.

Data parallel over 8 NeuronCores: each core handles 16384 rows.
"""

import numpy as np

B = 131072
D = 512
NCOMP = 6
HYPER = 2100.0
NCORES = 8
BL = B // NCORES          # 16384 rows per core
P = 128                   # partitions / rows per output tile
NV = 60                   # dictionary values per component
ND = NCOMP * NV           # 360 dictionary rows
CK = 120                  # dictionary rows per chunk (2 components)
NCHUNK = ND // CK         # 3
R = 128                   # factorization rank (zero-padded)
GROUP = 4                 # output tiles per group (512 batch cols)

_CACHE: dict = {}


def _build_nc(bl):
    import concourse.bacc as bacc
    import concourse.mybir as mybir
    from concourse import tile

    f32 = mybir.dt.float32
    f16 = mybir.dt.float16
    Alu = mybir.AluOpType

    nc = bacc.Bacc(trn_type="TRN2")
    dct = nc.dram_tensor("dct", [NCOMP, bl], f16, kind="ExternalInput").ap()
    rep = nc.dram_tensor("rep", [NCOMP, ND], f16, kind="ExternalInput").ap()
    vvd = nc.dram_tensor("vvd", [CK, 1], f32, kind="ExternalInput").ap()
    amd = nc.dram_tensor("amd", [CK, NCHUNK * R], f16, kind="ExternalInput").ap()
    bmd = nc.dram_tensor("bmd", [R, D], f16, kind="ExternalInput").ap()
    out = nc.dram_tensor("out", [bl, D], f32, kind="ExternalOutput").ap()

    ntiles = bl // P
    ngroups = ntiles // GROUP
    GB = GROUP * P            # 512 batch cols per group

    with tile.TileContext(nc) as tc:
        with (
            tc.tile_pool(name="const", bufs=1) as cpool,
            tc.tile_pool(name="oh", bufs=3) as ohpool,
            tc.tile_pool(name="gsb", bufs=2) as gpool,
            tc.tile_pool(name="osb", bufs=4) as opool,
            tc.tile_pool(name="crep", bufs=1, space="PSUM") as ppool,
            tc.tile_pool(name="gps", bufs=1, space="PSUM") as hpool,
            tc.tile_pool(name="ops", bufs=2, space="PSUM") as qpool,
        ):
            # ---- constants into SBUF.  dct is uploaded in chunks so the
            # prologue's first crep only waits for chunk 0 (~24 KB), not
            # the whole 196 KB (which takes ~7 us on this queue).
            NDC = 8
            DCW = bl // NDC
            dct_sb = cpool.tile([NCOMP, bl], f16, tag="dct")
            nc.sync.dma_start(out=dct_sb[:, 0:DCW], in_=dct[:, 0:DCW])
            rep_sb = cpool.tile([NCOMP, ND], f16, tag="rep")
            nc.sync.dma_start(out=rep_sb[:, :], in_=rep)
            vv_sb = cpool.tile([CK, 1], f32, tag="vv")
            nc.sync.dma_start(out=vv_sb[:, :], in_=vvd)
            a_sb = cpool.tile([CK, NCHUNK * R], f16, tag="amat")
            nc.sync.dma_start(out=a_sb[:, :], in_=amd)
            b_sb = cpool.tile([R, D], f16, tag="bmat")
            nc.sync.dma_start(out=b_sb[:, :], in_=bmd)
            for ch in range(1, NDC):
                nc.sync.dma_start(
                    out=dct_sb[:, ch * DCW:(ch + 1) * DCW],
                    in_=dct[:, ch * DCW:(ch + 1) * DCW],
                )

            # fixed PSUM tiles: crep 3 banks (one one-hot generation),
            # gp 1 bank, ops 2 pair-generations 4 banks.  Total 8.
            crep = ppool.tile([CK, NCHUNK * GB], f32, tag="crep")
            gp = hpool.tile([R, GB], f32, tag="gps")

            # HAM warm-up: ~5 us of back-to-back dummy matmuls so the PE
            # clock gate opens (K=8/8) before the real work; overlaps the
            # input DMAs.  Results are discarded (start=True overwrites).
            scr = cpool.tile([P, D], f16, tag="scr")
            nc.vector.memset(scr[:, :], 0.0)
            wps = qpool.tile([P, 2 * D], f32, tag="ops")
            for _ in range(9):
                nc.tensor.matmul(wps[:, 0:D], scr[:, 0:P], scr[:, :],
                                 start=True, stop=True)

            def filler(n):
                # dependency-free dummy matmuls that keep the PE activity
                # monitor fed through pipeline-fill holes
                for _ in range(n):
                    nc.tensor.matmul(wps[:, 0:D], scr[:, 0:P], scr[:, :],
                                     start=True, stop=True)

            def emit_crep3(g):
                for c in range(NCHUNK):
                    nc.tensor.matmul(
                        crep[:, c * GB:(c + 1) * GB],
                        rep_sb[:, c * CK:(c + 1) * CK],
                        dct_sb[:, g * GB:(g + 1) * GB],
                        start=True, stop=True,
                    )

            def emit_iseq_a(oh):
                nc.vector.tensor_scalar(
                    out=oh[:, 0:2 * GB], in0=crep[:, 0:2 * GB],
                    scalar1=vv_sb[:, :], scalar2=None, op0=Alu.is_equal,
                )

            def emit_iseq_b(oh):
                nc.vector.tensor_scalar(
                    out=oh[:, 2 * GB:3 * GB], in0=crep[:, 2 * GB:3 * GB],
                    scalar1=vv_sb[:, :], scalar2=None, op0=Alu.is_equal,
                )

            def emit_onehot(g):
                # replicate dct rows across the 120 partitions per chunk,
                # then two fused is_equal ops (c0+c1, then c2) so stage-1
                # can start ~0.75us before the last chunk's compare lands
                emit_crep3(g)
                oh = ohpool.tile([CK, NCHUNK * GB], f16, tag="oh")
                emit_iseq_a(oh)
                emit_iseq_b(oh)
                return oh

            # ---- prologue
            oh = emit_onehot(0)
            filler(4)

            for g in range(ngroups):
                # stage 1: G^T[r, 512b] over the 3 dict chunks
                for c in range(NCHUNK):
                    nc.tensor.matmul(
                        gp[:, :],
                        a_sb[:, c * R:(c + 1) * R],
                        oh[:, c * GB:(c + 1) * GB],
                        start=(c == 0), stop=(c == NCHUNK - 1),
                    )
                # next one-hot's replication + first compare, THEN the
                # G-copy, THEN the last compare: the DVE queue becomes
                # [iseq_a(g+1), gcopy(g), iseq_b(g+1)], so the next
                # one-hot starts immediately after crep instead of
                # waiting 0.7us behind the G-copy.
                g_sb = gpool.tile([R, GB], f16, tag="gsb")
                if g + 1 < ngroups:
                    emit_crep3(g + 1)
                    ohn = ohpool.tile([CK, NCHUNK * GB], f16, tag="oh")
                    emit_iseq_a(ohn)
                    nc.vector.tensor_copy(out=g_sb[:, :], in_=gp[:, :])
                    emit_iseq_b(ohn)
                    oh = ohn
                else:
                    nc.vector.tensor_copy(out=g_sb[:, :], in_=gp[:, :])
                # fillers cover the PE gap while gcopy runs on the DVE
                filler(3)
                # stage 2: 4 out tiles, K=R single pass, 2 psum bank-pairs
                pss = []
                for h in range(2):
                    ps = qpool.tile([P, 2 * D], f32, tag="ops")
                    for i in range(2):
                        t = 2 * h + i
                        nc.tensor.matmul(
                            ps[:, i * D:(i + 1) * D],
                            g_sb[:, t * P:(t + 1) * P],
                            b_sb[:, :],
                            start=True, stop=True,
                        )
                    pss.append(ps)
                # evacuate PSUM on ACT and ship
                ob = opool.tile([P, GROUP * D], f32, tag="ob")
                for h in range(2):
                    nc.scalar.mul(
                        ob[:, h * 2 * D:(h + 1) * 2 * D], pss[h][:, :], 1.0
                    )
                for t in range(GROUP):
                    r0 = (g * GROUP + t) * P
                    nc.sync.dma_start(
                        out=out[r0:r0 + P, :], in_=ob[:, t * D:(t + 1) * D]
                    )

    nc.compile()
    return nc


def _get_nc(bl=BL):
    key = ("nc", bl)
    if key not in _CACHE:
        _CACHE[key] = _build_nc(bl)
    return _CACHE[key]


def _host_factors(prm):
    """T = A @ Bm via SVD in fp64; zero-padded to rank R; 1/6 mean fold in T."""
    j = np.arange(0, D, 2, dtype=np.float64)
    invf = HYPER ** (-(2.0 * (j + 1.0)) / D)
    v = np.arange(NV, dtype=np.float64)[:, None]
    rows = []
    for k in range(NCOMP):
        ph = float(prm[k]) * v * invf[None, :]
        t = np.empty((NV, D))
        t[:, 0::2] = np.sin(ph) / NCOMP
        t[:, 1::2] = np.cos(ph) / NCOMP
        rows.append(t)
    T = np.concatenate(rows, 0)                      # [360, D]
    U, S, Vt = np.linalg.svd(T, full_matrices=False)
    r = min(R, S.shape[0])
    sq = np.sqrt(S[:r])
    A = U[:, :r] * sq[None, :]                       # [360, r]
    Bm = sq[:, None] * Vt[:r, :]                     # [r, D]
    # pack A chunk-major: amat[p, c*R + j] = A[c*CK + p, j]
    amat = np.zeros((CK, NCHUNK * R), np.float32)
    for c in range(NCHUNK):
        amat[:, c * R:c * R + r] = A[c * CK:(c + 1) * CK, :]
    bmat = np.zeros((R, D), np.float32)
    bmat[:r, :] = Bm
    return amat.astype(np.float16), bmat.astype(np.float16)


def _in_maps(date_components, params):
    dc = np.asarray(date_components).astype(np.int32, copy=False)
    prm = np.asarray(params).astype(np.float32, copy=False).reshape(NCOMP)
    amat, bmat = _host_factors(prm)
    # replication matrix: rep[k, d] = 1 if k == d // NV  (exact in fp16)
    rep = np.zeros((NCOMP, ND), np.float16)
    for k in range(NCOMP):
        rep[k, k * NV:(k + 1) * NV] = 1.0
    vv = (np.arange(CK) % NV).astype(np.float32).reshape(CK, 1)
    maps = []
    for i in range(NCORES):
        shard = dc[i * BL:(i + 1) * BL]
        dct = np.ascontiguousarray(shard.T).astype(np.float16)
        maps.append({
            "dct": dct,
            "rep": rep,
            "vvd": vv,
            "amd": amat,
            "bmd": bmat,
        })
    return maps


def kernel(date_components, params, _trace=False):
    from concourse.bass_utils import run_bass_kernel_spmd

    nc = _get_nc()
    maps = _in_maps(date_components, params)
    res = run_bass_kernel_spmd(
        nc, maps, core_ids=list(range(NCORES)),
        trace=_trace, trace_cores=[0] if _trace else None,
    )
    kernel.last_results = res
    return np.concatenate([r["out"] for r in res.results], axis=0)


# revision 31
# speedup vs baseline: 1.0333x; 1.0333x over previous
"""Trainium2 Bass kernel for nn_PositionalEmbedding (embedding-lookup form).

Math: out[b, 2j]   = mean_k sin(params[k] * dc[b,k] * inv_freq[j])
      out[b, 2j+1] = mean_k cos(params[k] * dc[b,k] * inv_freq[j])

dc[b,k] are integers in [0, 60), so the whole map is out = OH @ T where
OH[b, (k,v)] is a 360-col one-hot (6 ones per row) and T[(k,v), f] the
sin/cos dictionary.  T is numerically low rank (the small inv_freq columns
are near-polynomial in v), so the host factors T = A @ Bm (SVD, r=128,
tail ~1e-12) and the device computes

    G^T[r, b]  = sum_c A_c^T @ OH_c          (3 matmuls per 512-col group)
    out[b, f]  = (G^T slice)^T @ Bm          (4 matmuls, K=r, per group)

which runs the 360-deep contraction once per 512 batch columns instead of
once per 128-row tile (the baseline), cutting PE streaming ~40% and making
the kernel output-DMA bound (34 MB fp32 @ ~358 GB/s/core ≈ 94 us).

One-hot build per group: 3 replication matmuls (K=6) into 3 adjacent PSUM
banks + two fused DVE is_equal ops (chunks 0-1, then 2; the per-partition
scalar v = p%60 is the same for all chunks).  A ~5 us dummy-matmul warm-up
plus filler matmuls keep the PE HAM clock gate at K=8/8 throughout.

Data parallel over 8 NeuronCores: each core handles 16384 rows.
"""

import numpy as np

B = 131072
D = 512
NCOMP = 6
HYPER = 2100.0
NCORES = 8
BL = B // NCORES          # 16384 rows per core
P = 128                   # partitions / rows per output tile
NV = 60                   # dictionary values per component
ND = NCOMP * NV           # 360 dictionary rows
CK = 120                  # dictionary rows per chunk (2 components)
NCHUNK = ND // CK         # 3
R = 128                   # factorization rank (zero-padded)
GROUP = 4                 # output tiles per group (512 batch cols)

_CACHE: dict = {}


def _build_nc(bl):
    import concourse.bacc as bacc
    import concourse.mybir as mybir
    from concourse import tile

    f32 = mybir.dt.float32
    f16 = mybir.dt.float16
    Alu = mybir.AluOpType

    nc = bacc.Bacc(trn_type="TRN2")
    dct = nc.dram_tensor("dct", [NCOMP, bl], f16, kind="ExternalInput").ap()
    rep = nc.dram_tensor("rep", [NCOMP, ND], f16, kind="ExternalInput").ap()
    vvd = nc.dram_tensor("vvd", [CK, 1], f32, kind="ExternalInput").ap()
    amd = nc.dram_tensor("amd", [CK, NCHUNK * R], f16, kind="ExternalInput").ap()
    bmd = nc.dram_tensor("bmd", [R, D], f16, kind="ExternalInput").ap()
    out = nc.dram_tensor("out", [bl, D], f32, kind="ExternalOutput").ap()

    ntiles = bl // P
    ngroups = ntiles // GROUP
    GB = GROUP * P            # 512 batch cols per group

    with tile.TileContext(nc) as tc:
        with (
            tc.tile_pool(name="const", bufs=1) as cpool,
            tc.tile_pool(name="oh", bufs=3) as ohpool,
            tc.tile_pool(name="gsb", bufs=2) as gpool,
            tc.tile_pool(name="osb", bufs=4) as opool,
            tc.tile_pool(name="crep", bufs=1, space="PSUM") as ppool,
            tc.tile_pool(name="gps", bufs=1, space="PSUM") as hpool,
            tc.tile_pool(name="ops", bufs=2, space="PSUM") as qpool,
        ):
            # ---- constants into SBUF.  dct is uploaded in chunks so the
            # prologue's first crep only waits for chunk 0 (~24 KB), not
            # the whole 196 KB (which takes ~7 us on this queue).
            NDC = 8
            DCW = bl // NDC
            dct_sb = cpool.tile([NCOMP, bl], f16, tag="dct")
            nc.sync.dma_start(out=dct_sb[:, 0:DCW], in_=dct[:, 0:DCW])
            rep_sb = cpool.tile([NCOMP, ND], f16, tag="rep")
            nc.sync.dma_start(out=rep_sb[:, :], in_=rep)
            vv_sb = cpool.tile([CK, 1], f32, tag="vv")
            nc.sync.dma_start(out=vv_sb[:, :], in_=vvd)
            a_sb = cpool.tile([CK, NCHUNK * R], f16, tag="amat")
            nc.sync.dma_start(out=a_sb[:, :], in_=amd)
            b_sb = cpool.tile([R, D], f16, tag="bmat")
            nc.sync.dma_start(out=b_sb[:, :], in_=bmd)
            for ch in range(1, NDC):
                nc.sync.dma_start(
                    out=dct_sb[:, ch * DCW:(ch + 1) * DCW],
                    in_=dct[:, ch * DCW:(ch + 1) * DCW],
                )

            # fixed PSUM tiles: crep 3 banks (one one-hot generation),
            # gp 1 bank, ops 2 pair-generations 4 banks.  Total 8.
            crep = ppool.tile([CK, NCHUNK * GB], f32, tag="crep")
            gp = hpool.tile([R, GB], f32, tag="gps")

            # HAM warm-up: ~5 us of back-to-back dummy matmuls so the PE
            # clock gate opens (K=8/8) before the real work; overlaps the
            # input DMAs.  Results are discarded (start=True overwrites).
            scr = cpool.tile([P, D], f16, tag="scr")
            nc.vector.memset(scr[:, :], 0.0)
            wps = qpool.tile([P, 2 * D], f32, tag="ops")
            for _ in range(5):
                nc.tensor.matmul(wps[:, 0:D], scr[:, 0:P], scr[:, :],
                                 start=True, stop=True)

            def filler(n):
                # dependency-free dummy matmuls that keep the PE activity
                # monitor fed through pipeline-fill holes
                for _ in range(n):
                    nc.tensor.matmul(wps[:, 0:D], scr[:, 0:P], scr[:, :],
                                     start=True, stop=True)

            def emit_onehot(g):
                # replicate dct rows across the 120 partitions per chunk,
                # then two fused is_equal ops (c0+c1, then c2) so stage-1
                # can start ~0.75us before the last chunk's compare lands
                for c in range(NCHUNK):
                    nc.tensor.matmul(
                        crep[:, c * GB:(c + 1) * GB],
                        rep_sb[:, c * CK:(c + 1) * CK],
                        dct_sb[:, g * GB:(g + 1) * GB],
                        start=True, stop=True,
                    )
                oh = ohpool.tile([CK, NCHUNK * GB], f16, tag="oh")
                nc.vector.tensor_scalar(
                    out=oh[:, 0:2 * GB], in0=crep[:, 0:2 * GB],
                    scalar1=vv_sb[:, :], scalar2=None, op0=Alu.is_equal,
                )
                nc.vector.tensor_scalar(
                    out=oh[:, 2 * GB:3 * GB], in0=crep[:, 2 * GB:3 * GB],
                    scalar1=vv_sb[:, :], scalar2=None, op0=Alu.is_equal,
                )
                return oh

            # ---- prologue
            oh = emit_onehot(0)
            filler(5)

            for g in range(ngroups):
                # stage 1: G^T[r, 512b] over the 3 dict chunks
                for c in range(NCHUNK):
                    nc.tensor.matmul(
                        gp[:, :],
                        a_sb[:, c * R:(c + 1) * R],
                        oh[:, c * GB:(c + 1) * GB],
                        start=(c == 0), stop=(c == NCHUNK - 1),
                    )
                # G -> SBUF fp16 (stage-2 stationary operand).  Engine
                # alternates per group: on odd groups the DVE is freed to
                # start the next one-hot compare 0.7us earlier, without
                # saturating ACT every period.
                g_sb = gpool.tile([R, GB], f16, tag="gsb")
                if g % 2 == 0:
                    nc.vector.tensor_copy(out=g_sb[:, :], in_=gp[:, :])
                else:
                    nc.scalar.mul(g_sb[:, :], gp[:, :], 1.0)
                # next group's one-hot (PE: crep x3; DVE: 2 is_equal)
                if g + 1 < ngroups:
                    oh = emit_onehot(g + 1)
                # stage 2: 4 out tiles, K=R single pass, 2 psum bank-pairs
                pss = []
                for h in range(2):
                    ps = qpool.tile([P, 2 * D], f32, tag="ops")
                    for i in range(2):
                        t = 2 * h + i
                        nc.tensor.matmul(
                            ps[:, i * D:(i + 1) * D],
                            g_sb[:, t * P:(t + 1) * P],
                            b_sb[:, :],
                            start=True, stop=True,
                        )
                    pss.append(ps)
                filler(1)
                # evacuate PSUM on ACT and ship.  In the final group the
                # second pair goes to the (idle) DVE - different PSUM banks,
                # so the two copies run in parallel and shorten the drain.
                ob = opool.tile([P, GROUP * D], f32, tag="ob")
                nc.scalar.mul(ob[:, 0:2 * D], pss[0][:, :], 1.0)
                if g == ngroups - 1:
                    nc.vector.tensor_copy(out=ob[:, 2 * D:4 * D], in_=pss[1][:, :])
                else:
                    nc.scalar.mul(ob[:, 2 * D:4 * D], pss[1][:, :], 1.0)
                for t in range(GROUP):
                    r0 = (g * GROUP + t) * P
                    nc.sync.dma_start(
                        out=out[r0:r0 + P, :], in_=ob[:, t * D:(t + 1) * D]
                    )

    nc.compile()
    return nc


def _get_nc(bl=BL):
    key = ("nc", bl)
    if key not in _CACHE:
        _CACHE[key] = _build_nc(bl)
    return _CACHE[key]


def _host_factors(prm):
    """T = A @ Bm via SVD in fp64; zero-padded to rank R; 1/6 mean fold in T."""
    j = np.arange(0, D, 2, dtype=np.float64)
    invf = HYPER ** (-(2.0 * (j + 1.0)) / D)
    v = np.arange(NV, dtype=np.float64)[:, None]
    rows = []
    for k in range(NCOMP):
        ph = float(prm[k]) * v * invf[None, :]
        t = np.empty((NV, D))
        t[:, 0::2] = np.sin(ph) / NCOMP
        t[:, 1::2] = np.cos(ph) / NCOMP
        rows.append(t)
    T = np.concatenate(rows, 0)                      # [360, D]
    U, S, Vt = np.linalg.svd(T, full_matrices=False)
    r = min(R, S.shape[0])
    sq = np.sqrt(S[:r])
    A = U[:, :r] * sq[None, :]                       # [360, r]
    Bm = sq[:, None] * Vt[:r, :]                     # [r, D]
    # pack A chunk-major: amat[p, c*R + j] = A[c*CK + p, j]
    amat = np.zeros((CK, NCHUNK * R), np.float32)
    for c in range(NCHUNK):
        amat[:, c * R:c * R + r] = A[c * CK:(c + 1) * CK, :]
    bmat = np.zeros((R, D), np.float32)
    bmat[:r, :] = Bm
    return amat.astype(np.float16), bmat.astype(np.float16)


def _in_maps(date_components, params):
    dc = np.asarray(date_components).astype(np.int32, copy=False)
    prm = np.asarray(params).astype(np.float32, copy=False).reshape(NCOMP)
    amat, bmat = _host_factors(prm)
    # replication matrix: rep[k, d] = 1 if k == d // NV  (exact in fp16)
    rep = np.zeros((NCOMP, ND), np.float16)
    for k in range(NCOMP):
        rep[k, k * NV:(k + 1) * NV] = 1.0
    vv = (np.arange(CK) % NV).astype(np.float32).reshape(CK, 1)
    maps = []
    for i in range(NCORES):
        shard = dc[i * BL:(i + 1) * BL]
        dct = np.ascontiguousarray(shard.T).astype(np.float16)
        maps.append({
            "dct": dct,
            "rep": rep,
            "vvd": vv,
            "amd": amat,
            "bmd": bmat,
        })
    return maps


def kernel(date_components, params, _trace=False):
    from concourse.bass_utils import run_bass_kernel_spmd

    nc = _get_nc()
    maps = _in_maps(date_components, params)
    res = run_bass_kernel_spmd(
        nc, maps, core_ids=list(range(NCORES)),
        trace=_trace, trace_cores=[0] if _trace else None,
    )
    kernel.last_results = res
    return np.concatenate([r["out"] for r in res.results], axis=0)


# revision 32
# speedup vs baseline: 1.0371x; 1.0036x over previous
"""Trainium2 Bass kernel for nn_PositionalEmbedding (embedding-lookup form).

Math: out[b, 2j]   = mean_k sin(params[k] * dc[b,k] * inv_freq[j])
      out[b, 2j+1] = mean_k cos(params[k] * dc[b,k] * inv_freq[j])

dc[b,k] are integers in [0, 60), so the whole map is out = OH @ T where
OH[b, (k,v)] is a 360-col one-hot (6 ones per row) and T[(k,v), f] the
sin/cos dictionary.  T is numerically low rank (the small inv_freq columns
are near-polynomial in v), so the host factors T = A @ Bm (SVD, r=128,
tail ~1e-12) and the device computes

    G^T[r, b]  = sum_c A_c^T @ OH_c          (3 matmuls per 512-col group)
    out[b, f]  = (G^T slice)^T @ Bm          (4 matmuls, K=r, per group)

which runs the 360-deep contraction once per 512 batch columns instead of
once per 128-row tile (the baseline), cutting PE streaming ~40% and making
the kernel output-DMA bound (34 MB fp32 @ ~358 GB/s/core ≈ 94 us).

One-hot build per group: 3 replication matmuls (K=6) into 3 adjacent PSUM
banks + two fused DVE is_equal ops (chunks 0-1, then 2; the per-partition
scalar v = p%60 is the same for all chunks).  A ~5 us dummy-matmul warm-up
plus filler matmuls keep the PE HAM clock gate at K=8/8 throughout.

Data parallel over 8 NeuronCores: each core handles 16384 rows.
"""

import numpy as np

B = 131072
D = 512
NCOMP = 6
HYPER = 2100.0
NCORES = 8
BL = B // NCORES          # 16384 rows per core
P = 128                   # partitions / rows per output tile
NV = 60                   # dictionary values per component
ND = NCOMP * NV           # 360 dictionary rows
CK = 120                  # dictionary rows per chunk (2 components)
NCHUNK = ND // CK         # 3
R = 128                   # factorization rank (zero-padded)
GROUP = 4                 # output tiles per group (512 batch cols)

_CACHE: dict = {}


def _build_nc(bl):
    import concourse.bacc as bacc
    import concourse.mybir as mybir
    from concourse import tile

    f32 = mybir.dt.float32
    f16 = mybir.dt.float16
    Alu = mybir.AluOpType

    nc = bacc.Bacc(trn_type="TRN2")
    dct = nc.dram_tensor("dct", [NCOMP, bl], f16, kind="ExternalInput").ap()
    rep = nc.dram_tensor("rep", [NCOMP, ND], f16, kind="ExternalInput").ap()
    vvd = nc.dram_tensor("vvd", [CK, 1], f32, kind="ExternalInput").ap()
    amd = nc.dram_tensor("amd", [CK, NCHUNK * R], f16, kind="ExternalInput").ap()
    bmd = nc.dram_tensor("bmd", [R, D], f16, kind="ExternalInput").ap()
    out = nc.dram_tensor("out", [bl, D], f32, kind="ExternalOutput").ap()

    ntiles = bl // P
    ngroups = ntiles // GROUP
    GB = GROUP * P            # 512 batch cols per group

    with tile.TileContext(nc) as tc:
        with (
            tc.tile_pool(name="const", bufs=1) as cpool,
            tc.tile_pool(name="oh", bufs=3) as ohpool,
            tc.tile_pool(name="gsb", bufs=2) as gpool,
            tc.tile_pool(name="osb", bufs=4) as opool,
            tc.tile_pool(name="crep", bufs=1, space="PSUM") as ppool,
            tc.tile_pool(name="gps", bufs=1, space="PSUM") as hpool,
            tc.tile_pool(name="ops", bufs=2, space="PSUM") as qpool,
        ):
            # ---- constants into SBUF.  dct is uploaded in chunks so the
            # prologue's first crep only waits for chunk 0 (~24 KB), not
            # the whole 196 KB (which takes ~7 us on this queue).
            NDC = 8
            DCW = bl // NDC
            dct_sb = cpool.tile([NCOMP, bl], f16, tag="dct")
            nc.sync.dma_start(out=dct_sb[:, 0:DCW], in_=dct[:, 0:DCW])
            rep_sb = cpool.tile([NCOMP, ND], f16, tag="rep")
            nc.sync.dma_start(out=rep_sb[:, :], in_=rep)
            vv_sb = cpool.tile([CK, 1], f32, tag="vv")
            nc.sync.dma_start(out=vv_sb[:, :], in_=vvd)
            a_sb = cpool.tile([CK, NCHUNK * R], f16, tag="amat")
            nc.sync.dma_start(out=a_sb[:, :], in_=amd)
            b_sb = cpool.tile([R, D], f16, tag="bmat")
            nc.sync.dma_start(out=b_sb[:, :], in_=bmd)
            for ch in range(1, NDC):
                nc.sync.dma_start(
                    out=dct_sb[:, ch * DCW:(ch + 1) * DCW],
                    in_=dct[:, ch * DCW:(ch + 1) * DCW],
                )

            # fixed PSUM tiles: crep 3 banks (one one-hot generation),
            # gp 1 bank, ops 2 pair-generations 4 banks.  Total 8.
            crep = ppool.tile([CK, NCHUNK * GB], f32, tag="crep")
            gp = hpool.tile([R, GB], f32, tag="gps")

            # HAM warm-up: ~5 us of back-to-back dummy matmuls so the PE
            # clock gate opens (K=8/8) before the real work; overlaps the
            # input DMAs.  Results are discarded (start=True overwrites).
            scr = cpool.tile([P, D], f16, tag="scr")
            nc.vector.memset(scr[:, :], 0.0)
            wps = qpool.tile([P, 2 * D], f32, tag="ops")
            for _ in range(5):
                nc.tensor.matmul(wps[:, 0:D], scr[:, 0:P], scr[:, :],
                                 start=True, stop=True)

            def filler(n):
                # dependency-free dummy matmuls that keep the PE activity
                # monitor fed through pipeline-fill holes
                for _ in range(n):
                    nc.tensor.matmul(wps[:, 0:D], scr[:, 0:P], scr[:, :],
                                     start=True, stop=True)

            def emit_onehot(g):
                # replicate dct rows across the 120 partitions per chunk,
                # then two fused is_equal ops (c0+c1, then c2) so stage-1
                # can start ~0.75us before the last chunk's compare lands
                for c in range(NCHUNK):
                    nc.tensor.matmul(
                        crep[:, c * GB:(c + 1) * GB],
                        rep_sb[:, c * CK:(c + 1) * CK],
                        dct_sb[:, g * GB:(g + 1) * GB],
                        start=True, stop=True,
                    )
                oh = ohpool.tile([CK, NCHUNK * GB], f16, tag="oh")
                nc.vector.tensor_scalar(
                    out=oh[:, 0:2 * GB], in0=crep[:, 0:2 * GB],
                    scalar1=vv_sb[:, :], scalar2=None, op0=Alu.is_equal,
                )
                nc.vector.tensor_scalar(
                    out=oh[:, 2 * GB:3 * GB], in0=crep[:, 2 * GB:3 * GB],
                    scalar1=vv_sb[:, :], scalar2=None, op0=Alu.is_equal,
                )
                return oh

            # ---- prologue
            oh = emit_onehot(0)
            filler(5)

            for g in range(ngroups):
                # stage 1: G^T[r, 512b] over the 3 dict chunks
                for c in range(NCHUNK):
                    nc.tensor.matmul(
                        gp[:, :],
                        a_sb[:, c * R:(c + 1) * R],
                        oh[:, c * GB:(c + 1) * GB],
                        start=(c == 0), stop=(c == NCHUNK - 1),
                    )
                # G -> SBUF fp16 (stage-2 stationary operand).  Engine
                # alternates per group: on odd groups the DVE is freed to
                # start the next one-hot compare 0.7us earlier, without
                # saturating ACT every period.
                g_sb = gpool.tile([R, GB], f16, tag="gsb")
                if g % 2 == 0:
                    nc.vector.tensor_copy(out=g_sb[:, :], in_=gp[:, :])
                else:
                    nc.scalar.mul(g_sb[:, :], gp[:, :], 1.0)
                # next group's one-hot (PE: crep x3; DVE: 2 is_equal)
                if g + 1 < ngroups:
                    oh = emit_onehot(g + 1)
                # stage 2: 4 out tiles, K=R single pass, 2 psum bank-pairs
                pss = []
                for h in range(2):
                    ps = qpool.tile([P, 2 * D], f32, tag="ops")
                    for i in range(2):
                        t = 2 * h + i
                        nc.tensor.matmul(
                            ps[:, i * D:(i + 1) * D],
                            g_sb[:, t * P:(t + 1) * P],
                            b_sb[:, :],
                            start=True, stop=True,
                        )
                    pss.append(ps)
                filler(1)
                # evacuate PSUM on ACT and ship.  In the final group the
                # second pair goes to the (idle) DVE - different PSUM banks,
                # so the two copies run in parallel and shorten the drain.
                ob = opool.tile([P, GROUP * D], f32, tag="ob")
                nc.scalar.mul(ob[:, 0:2 * D], pss[0][:, :], 1.0)
                if g == ngroups - 1:
                    nc.vector.tensor_copy(out=ob[:, 2 * D:4 * D], in_=pss[1][:, :])
                else:
                    nc.scalar.mul(ob[:, 2 * D:4 * D], pss[1][:, :], 1.0)
                # 2 fused DMAs per group: DRAM ap rearranged to
                # (p, t, c) so one instruction ships a 512 KB tile-pair
                for h in range(2):
                    r0 = (g * GROUP + 2 * h) * P
                    dap = out[r0:r0 + 2 * P, :].rearrange(
                        "(t p) c -> p t c", t=2
                    )
                    nc.sync.dma_start(
                        out=dap, in_=ob[:, h * 2 * D:(h + 1) * 2 * D]
                    )

    nc.compile()
    return nc


def _get_nc(bl=BL):
    key = ("nc", bl)
    if key not in _CACHE:
        _CACHE[key] = _build_nc(bl)
    return _CACHE[key]


def _host_factors(prm):
    """T = A @ Bm via SVD in fp64; zero-padded to rank R; 1/6 mean fold in T."""
    j = np.arange(0, D, 2, dtype=np.float64)
    invf = HYPER ** (-(2.0 * (j + 1.0)) / D)
    v = np.arange(NV, dtype=np.float64)[:, None]
    rows = []
    for k in range(NCOMP):
        ph = float(prm[k]) * v * invf[None, :]
        t = np.empty((NV, D))
        t[:, 0::2] = np.sin(ph) / NCOMP
        t[:, 1::2] = np.cos(ph) / NCOMP
        rows.append(t)
    T = np.concatenate(rows, 0)                      # [360, D]
    U, S, Vt = np.linalg.svd(T, full_matrices=False)
    r = min(R, S.shape[0])
    sq = np.sqrt(S[:r])
    A = U[:, :r] * sq[None, :]                       # [360, r]
    Bm = sq[:, None] * Vt[:r, :]                     # [r, D]
    # pack A chunk-major: amat[p, c*R + j] = A[c*CK + p, j]
    amat = np.zeros((CK, NCHUNK * R), np.float32)
    for c in range(NCHUNK):
        amat[:, c * R:c * R + r] = A[c * CK:(c + 1) * CK, :]
    bmat = np.zeros((R, D), np.float32)
    bmat[:r, :] = Bm
    return amat.astype(np.float16), bmat.astype(np.float16)


def _in_maps(date_components, params):
    dc = np.asarray(date_components).astype(np.int32, copy=False)
    prm = np.asarray(params).astype(np.float32, copy=False).reshape(NCOMP)
    amat, bmat = _host_factors(prm)
    # replication matrix: rep[k, d] = 1 if k == d // NV  (exact in fp16)
    rep = np.zeros((NCOMP, ND), np.float16)
    for k in range(NCOMP):
        rep[k, k * NV:(k + 1) * NV] = 1.0
    vv = (np.arange(CK) % NV).astype(np.float32).reshape(CK, 1)
    maps = []
    for i in range(NCORES):
        shard = dc[i * BL:(i + 1) * BL]
        dct = np.ascontiguousarray(shard.T).astype(np.float16)
        maps.append({
            "dct": dct,
            "rep": rep,
            "vvd": vv,
            "amd": amat,
            "bmd": bmat,
        })
    return maps


def kernel(date_components, params, _trace=False):
    from concourse.bass_utils import run_bass_kernel_spmd

    nc = _get_nc()
    maps = _in_maps(date_components, params)
    res = run_bass_kernel_spmd(
        nc, maps, core_ids=list(range(NCORES)),
        trace=_trace, trace_cores=[0] if _trace else None,
    )
    kernel.last_results = res
    return np.concatenate([r["out"] for r in res.results], axis=0)
